# revision 23
# baseline (speedup 1.0000x reference)
"""Multi-head self-attention Trainium2 kernel (8 NeuronCores).

Problem: B=4, N=2048, D=1024, H=16 heads of dim 64, fp32 in/out.

Sharding: 8 cores = 4 batches x 2 head-groups. Core c handles batch c//2
and heads (c%2)*8 .. (c%2)*8+7 (a 512-wide slice of the hidden dim).
Each core computes q/k/v projections for its head slice, attention for
its 8 heads, and a partial out-projection (contraction over its 512
attention dims). Host sums the two partials per batch.

Device dataflow (per core), all matmuls bf16 with fp32 PSUM accumulate:
  - x^T (host-pretransposed, bf16) lives in SBUF as 8 [128, 2048] tiles.
  - q_a/k_a = W^T.T @ x^T in "layout a" [head_dim-part, token-free].
  - v in "layout b" [token-part, head_dim-free], restrided into per-head
    65-column segments whose last column is ones (gives the softmax
    denominator for free during the PV matmul).
  - scores computed transposed: S^T[j, i] = k_a^T q_a (K=64 contraction),
    exp on ScalarE (scale=1/8 folded in, no max subtraction -- scores are
    ~N(0,1) so exp is safe), output P^T bf16 straight to SBUF.
  - PV: out[65, i] += v'[j,:65]^T P^T[j, i]; row 64 = sum_j exp = denom.
  - normalize: reciprocal_approx_fast of the denominator row (DVE, ~5x
    the exact DVE reciprocal), gpsimd partition-broadcast, DVE multiply
    into per-(pair, i-block) attn tiles (separate tiles per i-block so
    out-projection reads never serialize against later normalizes at
    tile granularity).
  - out-projection: o[token, d_out] = attn^T.T @ Wo_slice^T, fp32 out.

Schedule notes (measured on HW): the PE issues back-to-back matmuls at
~222ns/512-col when the stationary config is unchanged; every
scores<->PV stationary-shape switch costs ~100-200ns (the LDWEIGHTS of
a different config cannot overlap the running matmul), and a PE stall
also drops the PE p-state clock for the next few matmuls. The exp
stream (ACT engine) is ~285us busy; PE stream ~341us. PSUM (8 banks)
only fits 2 scores tiles + 2 PV accumulators, so scores(j+1) recycles
the psum freed by exp(j-1): the j-loop software-pipelines PV three
steps behind scores (LAG=3) and tops every unit up with independent PE
filler work (v-projection just-in-time in unit 0, later pairs' k/q
projections in the lead-in and units 1-3, icb0 out-projections in
units 4-7) so the PE never waits on the exp latency. Weights arrive as
one host-packed DMA per tensor (dma_start dispatch costs ~630ns each
on the sync queue). The last unit's normalize reads PV straight from
PSUM to shorten the tail chain before the final out-projections.

Biases: bq applied on device (per-partition in layout a). bk cancels
exactly in softmax (adds a per-query constant to scores). bv and bo are
folded on host: attn rows sum to 1 so bv passes through linearly.
"""

import numpy as np
import ml_dtypes

BF16 = ml_dtypes.bfloat16

HIDDEN = 1024
N_TOK = 2048
BATCH = 4
N_CORES = 8

_CACHE = {}


def _build_nc(D, N):
    """Build + compile the per-core Bass program.

    Per-core tensor shapes (DL = D // 2 local q/k/v width):
      xT  [D, N]  bf16   : x[b] transposed
      wqT/wkT/wvT [D, DL] bf16 : W[hs:hs+DL, :].T
      woT [DL, D] bf16   : Wo[:, hs:hs+DL].T
      bqt [128, DL//128] f32 : bq slice, chunked per partition
      o   [N, D]  f32    : partial output (host sums pairs)
    """
    import concourse.bacc as bacc
    import concourse.mybir as mybir
    import concourse.tile as tile
    from contextlib import ExitStack

    dt = mybir.dt
    P = 128
    DL = D // 2
    KC = D // P          # d_model chunks (8)
    MC = DL // P         # head-dim chunks == head pairs (4)
    NT = N // P          # token tiles (16)
    ICB = N // 2         # i-block width (1024)
    MMW = min(512, ICB)  # matmul moving width
    JT = NT              # j tiles (16)

    nc = bacc.Bacc("TRN2", target_bir_lowering=False, debug=False)

    xT = nc.dram_tensor("xT", [D, N], dt.bfloat16, kind="ExternalInput")
    wqT = nc.dram_tensor("wqT", [P, KC * DL], dt.bfloat16, kind="ExternalInput")
    wkT = nc.dram_tensor("wkT", [P, KC * DL], dt.bfloat16, kind="ExternalInput")
    wvT = nc.dram_tensor("wvT", [P, KC * DL], dt.bfloat16, kind="ExternalInput")
    woT = nc.dram_tensor("woT", [P, MC * D], dt.bfloat16, kind="ExternalInput")
    bqt = nc.dram_tensor("bqt", [P, MC], dt.float32, kind="ExternalInput")
    o = nc.dram_tensor("o", [N, D], dt.float32, kind="ExternalOutput")

    with tile.TileContext(nc) as tc, ExitStack() as ctx:
        pers = ctx.enter_context(tc.tile_pool(name="pers", bufs=1))
        work = ctx.enter_context(tc.tile_pool(name="work", bufs=2))
        pmm = ctx.enter_context(tc.tile_pool(name="pmm", bufs=2, space="PSUM"))
        ppv = ctx.enter_context(tc.tile_pool(name="ppv", bufs=2, space="PSUM"))

        # ---- persistent SBUF tiles + input DMAs ----
        # Weights arrive host-packed as [128, chunks*width] so each
        # tensor is ONE big DMA: the sync engine's ~630ns-per-dma_start
        # dispatch cost made 41 small input DMAs cost ~25us of lead-in.
        # xT stays chunked so the first projections can consume it as it
        # streams.
        xt_t = [pers.tile([P, N], dt.bfloat16, name=f"xT{k}", tag=f"xT{k}") for k in range(KC)]
        wqb = pers.tile([P, KC * DL], dt.bfloat16, name="wqb", tag="wqb")
        wkb = pers.tile([P, KC * DL], dt.bfloat16, name="wkb", tag="wkb")
        wvb = pers.tile([P, KC * DL], dt.bfloat16, name="wvb", tag="wvb")
        wob = pers.tile([P, MC * D], dt.bfloat16, name="wob", tag="wob")
        bq_t = pers.tile([P, MC], dt.float32, name="bqt_sb", tag="bqt")
        qa = [pers.tile([P, N], dt.bfloat16, name=f"qa{m}", tag=f"qa{m}") for m in range(MC)]
        ka = [pers.tile([P, N], dt.bfloat16, name=f"ka{m}", tag=f"ka{m}") for m in range(MC)]
        vp = [pers.tile([P, 8 * 65], dt.bfloat16, name=f"vp{t}", tag=f"vp{t}") for t in range(NT)]
        attn = [[pers.tile([P, ICB], dt.bfloat16, name=f"attn{m}_{i}",
                           tag=f"attn{m}_{i}") for i in range(2)]
                for m in range(MC)]

        HW = KC * DL // 2
        nc.sync.dma_start(bq_t[:], bqt[:, :])
        nc.sync.dma_start(wkb[:, 0:HW], wkT[:, 0:HW])
        nc.sync.dma_start(xt_t[0][:], xT[0:P, :])
        nc.sync.dma_start(wkb[:, HW:], wkT[:, HW:])
        nc.sync.dma_start(xt_t[1][:], xT[P:2 * P, :])
        nc.sync.dma_start(wqb[:, 0:HW], wqT[:, 0:HW])
        nc.sync.dma_start(xt_t[2][:], xT[2 * P:3 * P, :])
        nc.sync.dma_start(wqb[:, HW:], wqT[:, HW:])
        for k in range(3, KC):
            nc.sync.dma_start(xt_t[k][:], xT[k * P:(k + 1) * P, :])
        nc.sync.dma_start(wvb[:], wvT[:, :])
        nc.sync.dma_start(wob[:], woT[:, :])

        # PE p-state warm-up: the PE clock ramps (0.65 -> 1.2 -> 2.4 GHz
        # over ~3us of continuous execution), so burn tiny matmuls on the
        # already-arrived bias tile while the xT DMA streams, so the real
        # projections start at full clock.
        wps = pmm.tile([P, MC], dt.float32, tag="mm", name="warm")
        for _ in range(40):
            nc.tensor.matmul(out=wps[0:MC, 0:MC], lhsT=bq_t[:, 0:MC],
                             rhs=bq_t[:, 0:MC], start=True, stop=True)

        def v_tile(t):
            # v projection for token tile t: [token, DL] restrided into
            # per-head 65-col segments with a trailing ones column.
            ps = pmm.tile([P, DL], dt.float32, tag="mm", name="psv")
            for k in range(KC):
                nc.tensor.matmul(
                    out=ps[:, 0:DL],
                    lhsT=xt_t[k][:, t * P:(t + 1) * P],
                    rhs=wvb[:, k * DL:(k + 1) * DL],
                    start=(k == 0),
                    stop=(k == KC - 1),
                )
            seg = vp[t][:].rearrange("p (s c) -> p s c", c=65)
            nc.vector.memset(seg[:, :, 64:65], 1.0)
            nc.vector.tensor_copy(
                seg[:, :, 0:64],
                ps[:].rearrange("p (s c) -> p s c", c=64),
            )

        def kq_group(m, which, n2):
            """One [128, ICB] projection block: k or q for pair m, token
            columns n2..n2+ICB."""
            wt, dst = (wkb, ka) if which == "k" else (wqb, qa)
            ps = pmm.tile([P, ICB], dt.float32, tag="mm", name="psp")
            for k in range(KC):
                for s in range(0, ICB, MMW):
                    nc.tensor.matmul(
                        out=ps[:, s:s + MMW],
                        lhsT=wt[:, k * DL + m * P:k * DL + (m + 1) * P],
                        rhs=xt_t[k][:, n2 + s:n2 + s + MMW],
                        start=(k == 0),
                        stop=(k == KC - 1),
                    )
            if which == "k":
                nc.vector.tensor_copy(dst[m][:, n2:n2 + ICB], ps[:])
            else:
                nc.vector.tensor_scalar_add(
                    dst[m][:, n2:n2 + ICB], ps[:], bq_t[:, m:m + 1])

        def attn_unit(m, ib, fillers=(), jit_v=False, direct_norm=False):
            """Attention for head pair (2m, 2m+1) over i-block ib.

            The j-loop is software-pipelined: PV(j-1) is emitted after
            scores(j), so the PE never sits on the in-order PV matmul
            waiting for exp(j) -- by the time PV(j-1) issues, exp(j-1)
            finished under scores(j). `fillers` is a list of (j, fn) PE
            work emitted at step j, topping units up to the ~35.6us exp
            stream so the exp latency never becomes a PE wait. With
            jit_v, v-projection tile j+1 is emitted at step j (tile 0 is
            pre-emitted by the caller). Returns the unnormalized PV
            results copied to SBUF fp32.
            """
            i0 = ib * ICB
            heads = (2 * m, 2 * m + 1)
            fillers = list(fillers)
            pvs = {}
            for h in heads:
                pvs[h] = ppv.tile([65, ICB], dt.float32, tag="pv", name="pv")
            lagged = []
            LAG = 3

            def pv_step(j, pts):
                for h in heads:
                    for s in range(0, ICB, MMW):
                        nc.tensor.matmul(
                            out=pvs[h][:, s:s + MMW],
                            lhsT=vp[j][:, h * 65:(h + 1) * 65],
                            rhs=pts[h][:, s:s + MMW],
                            start=(j == 0),
                            stop=(j == JT - 1),
                        )

            for j in range(JT):
                pss = {}
                for h in heads:
                    r = (h % 2) * 64
                    ps = pmm.tile([P, ICB], dt.float32, tag="mm", name="pss")
                    pss[h] = ps
                    for s in range(0, ICB, MMW):
                        nc.tensor.matmul(
                            out=ps[:, s:s + MMW],
                            lhsT=ka[m][r:r + 64, j * P:(j + 1) * P],
                            rhs=qa[m][r:r + 64, i0 + s:i0 + s + MMW],
                            start=True,
                            stop=True,
                        )
                pts = {}
                for h in heads:
                    pt = work.tile([P, ICB], dt.bfloat16, tag="pt",
                                   name="pt", bufs=12)
                    pts[h] = pt
                    nc.scalar.activation(
                        pt[:], pss[h][:],
                        mybir.ActivationFunctionType.Exp,
                        bias=0.0, scale=0.125,
                    )
                lagged.append((j, pts))
                if j % 2 == 1 and len(lagged) > LAG:
                    pv_step(*lagged.pop(0))
                    pv_step(*lagged.pop(0))
                if jit_v and j < JT - 1:
                    v_tile(j + 1)
                while fillers and fillers[0][0] <= j:
                    fillers.pop(0)[1]()
            for jp in lagged:
                pv_step(*jp)
            for _, f in fillers:  # any stragglers
                f()
            out = {}
            for h in heads:
                # hd rows and the denominator row go to separate tiles:
                # reciprocal_approx_fast (custom DVE op) only reads
                # correctly from a base-partition-0 AP. The last unit's
                # normalize reads straight from PSUM instead (shorter
                # chain off the end of the exp stream -- PSUM is free to
                # hold then).
                den = work.tile([1, ICB], dt.float32, tag="den",
                                name="den", bufs=3)
                nc.vector.tensor_copy(den[:], pvs[h][64:65, :])
                if direct_norm:
                    out[h] = (pvs[h][0:64, :], den)
                else:
                    pv_sb = work.tile([64, ICB], dt.float32, tag="pvsb",
                                      name="pvsb", bufs=3)
                    nc.vector.tensor_copy(pv_sb[:], pvs[h][0:64, :])
                    out[h] = (pv_sb, den)
            return (m, ib, out)

        def normalize(pending):
            """Softmax normalization, deferred one unit off the critical
            path. reciprocal_approx_fast (~18 bits, 5x faster than the
            exact DVE reciprocal) on the denominator row, partition
            broadcast + multiply on the otherwise-idle Pool engine."""
            m, ib, pv_sbs = pending
            steps = []
            for h, (pv_sb, den) in pv_sbs.items():
                recip = work.tile([1, ICB], dt.float32, tag="recip",
                                  name="recip")
                nc.vector.reciprocal_approx_fast(recip[:], den[:])
                bcast = work.tile([64, ICB], dt.float32, tag="bcast",
                                  name="bcast")
                steps.append((h, pv_sb, recip, bcast))
            for h, pv_sb, recip, bcast in steps:
                nc.gpsimd.partition_broadcast(bcast[:], recip[:])
            for h, pv_sb, recip, bcast in steps:
                nc.vector.tensor_tensor(
                    attn[m][ib][(h % 2) * 64:(h % 2) * 64 + 64, :],
                    pv_sb[:],
                    bcast[:],
                    mybir.AluOpType.mult,
                )

        def outproj(t, korder):
            ib = t // (NT // 2)
            tl = t % (NT // 2)
            ps = pmm.tile([P, D], dt.float32, tag="mm", name="pso")
            for ki, k in enumerate(korder):
                for s in range(0, D, 512):
                    w = min(512, D - s)
                    nc.tensor.matmul(
                        out=ps[:, s:s + w],
                        lhsT=attn[k][ib][:, tl * P:(tl + 1) * P],
                        rhs=wob[:, k * D + s:k * D + s + w],
                        start=(ki == 0),
                        stop=(ki == MC - 1),
                    )
            oe = work.tile([P, D], dt.float32, tag="oev", name="oe")
            nc.vector.tensor_copy(oe[:], ps[:])
            nc.sync.dma_start(o[t * P:(t + 1) * P, :], oe[:])

        # ---- schedule ----
        order = list(range(MC))
        units = [(m, 0) for m in order] + [(m, 1) for m in order]

        # Lead-in: just enough of pair 0's projections to start its ib0
        # attention (k over all tokens, q over ib0 tokens).
        kq_group(order[0], "k", 0)
        kq_group(order[0], "q", 0)
        kq_group(order[0], "k", ICB)
        kq_group(order[1], "k", 0)
        kq_group(order[1], "q", 0)
        kq_group(order[1], "k", ICB)
        v_tile(0)

        # PE filler budget: each attention unit has ~28.4us of its own PE
        # work against a ~35.6us exp stream, so ~7us of independent PE
        # work per unit keeps the PE (the overall bottleneck) from ever
        # waiting on exp. Unit 0 is oversubscribed by the jit
        # v-projection; units 1-2 stage the next pair's projections, unit
        # 3 the remaining q blocks, units 4-6 the icb0 out-projections
        # (all ib0 normalizes have landed by unit 4's start).
        def kqf(m_, w_, n_):
            return lambda: kq_group(m_, w_, n_)

        def opf(t_):
            return lambda: outproj(t_, order)

        fill = {i: [] for i in range(len(units))}
        fill[1] = [(2, kqf(order[2], "k", 0)),
                   (6, kqf(order[2], "q", 0)),
                   (9, kqf(order[2], "k", ICB)),
                   (12, kqf(order[0], "q", ICB))]
        fill[2] = [(2, kqf(order[3], "k", 0)),
                   (6, kqf(order[3], "q", 0)),
                   (9, kqf(order[3], "k", ICB)),
                   (12, kqf(order[1], "q", ICB))]
        fill[3] = [(4, kqf(order[2], "q", ICB)),
                   (10, kqf(order[3], "q", ICB))]
        fill[4] = [(9, opf(0)), (13, opf(1))]
        fill[5] = [(4, opf(2)), (10, opf(3))]
        fill[6] = [(4, opf(4)), (10, opf(5))]
        fill[7] = [(4, opf(6)), (10, opf(7))]

        pending = []
        for ui, (m, ib) in enumerate(units):
            if pending:
                normalize(pending.pop(0))
            pending.append(
                attn_unit(m, ib, fillers=fill[ui], jit_v=(ui == 0),
                          direct_norm=(ui == len(units) - 1)))
        # Tail: last unit's normalize on the DVE (shorter chain), then the
        # icb1 out-projections; korder ends on the last-normalized pair so
        # most of each tile's contraction overlaps the normalize.
        normalize(pending.pop(0))
        for t in range(NT // 2, NT):
            outproj(t, order)

    nc.compile()
    return nc


def _get_nc(D, N):
    key = (D, N)
    if key not in _CACHE:
        _CACHE[key] = _build_nc(D, N)
    return _CACHE[key]


def _make_in_maps(x, Wq, bq, Wk, Wv, Wo, D, N):
    DL = D // 2
    MC = DL // 128
    KC = D // 128

    def chunkpack(a):
        # [C*128, W] -> [128, C*W]: SBUF chunk c at free cols c*W..(c+1)*W
        c128, w = a.shape
        c = c128 // 128
        return np.ascontiguousarray(
            a.reshape(c, 128, w).transpose(1, 0, 2).reshape(128, c * w))

    in_maps = []
    for c in range(N_CORES):
        b = c // 2
        hs = (c % 2) * DL
        in_maps.append({
            "xT": np.ascontiguousarray(x[b].T).astype(BF16),
            "wqT": chunkpack(Wq[hs:hs + DL, :].T).astype(BF16),
            "wkT": chunkpack(Wk[hs:hs + DL, :].T).astype(BF16),
            "wvT": chunkpack(Wv[hs:hs + DL, :].T).astype(BF16),
            "woT": chunkpack(Wo[:, hs:hs + DL].T).astype(BF16),
            "bqt": np.ascontiguousarray(
                bq[hs:hs + DL].reshape(MC, 128).T).astype(np.float32),
        })
    return in_maps


def _run(x, Wq, bq, Wk, bk, Wv, bv, Wo, bo, trace=False):
    from concourse.bass_utils import run_bass_kernel_spmd

    x = np.asarray(x, np.float32)
    B, N, D = x.shape
    nc = _get_nc(D, N)
    in_maps = _make_in_maps(
        x, np.asarray(Wq, np.float32), np.asarray(bq, np.float32),
        np.asarray(Wk, np.float32), np.asarray(Wv, np.float32),
        np.asarray(Wo, np.float32), D, N)
    res = run_bass_kernel_spmd(
        nc, in_maps, list(range(N_CORES)), trace=trace)

    bv = np.asarray(bv, np.float32)
    bo = np.asarray(bo, np.float32)
    extra = bv @ np.asarray(Wo, np.float32).T + bo  # exact linear fold
    out = np.empty((B, N, D), np.float32)
    for b in range(B):
        out[b] = res.results[2 * b]["o"] + res.results[2 * b + 1]["o"] + extra
    return out, res


def kernel(x, Wq, bq, Wk, bk, Wv, bv, Wo, bo):
    out, _ = _run(x, Wq, bq, Wk, bk, Wv, bv, Wo, bo, trace=False)
    return out
